# revision 5
# baseline (speedup 1.0000x reference)
"""Trainium2 Bass kernel for the gated-attention nn.Module.

Math (per batch element b):
    deg   = rel_pos.sum(-1)                        # [N]
    gate  = sigmoid(deg * W_d + b_d)               # [N, D]
    xg    = x * gate
    qkv   = xg @ W_qkv.T + b_qkv                   # [N, 3D]
    qk, value, res = split(qkv); qk = sigmoid(qk)
    attn  = (qk @ qk.T) * scale * rel_pos          # [N, N]
    attn  = attn / (attn.sum(-1, keepdims) + 1e-6)
    out   = relu(attn @ value + res)               # [N, D]

Sharding: pure data-parallel over batch, B == 8 == n_cores, one batch
element per NeuronCore, no collectives.

Per-core layout strategy: rel_pos is streamed once from HBM (fp32,
cast to bf16 by the SWDGE), row-sums reduced for deg, and a transposed
bf16 copy (RT = rel_pos.T) is built on-chip with the DMA XBAR so the
attention matrix can be produced directly in [m, n] (key-major)
orientation.  That orientation makes the normalizer a free extra
matmul column (ones appended to value) and needs no PE transposes of
the N x N matrix.
"""

import math
from contextlib import ExitStack

import numpy as np

import concourse.bass as bass
import concourse.tile as tile
from concourse import bacc, mybir
from concourse.bass import ts
from concourse.bass_utils import run_bass_kernel_spmd
from concourse.masks import make_identity

B, N, D = 8, 2048, 256
E = 3 * D  # 768
NT = N // 128  # 16 row tiles
DC = D // 128  # 2 dim chunks
NG = 4  # output-tile groups (each covers 512 query rows)
GT = NT // NG  # out tiles per group (4)
SCALE = 1.0 / math.sqrt(32.0)
EPS = 1e-6

F32 = mybir.dt.float32
BF16 = mybir.dt.bfloat16

AL = mybir.AluOpType
AF = mybir.ActivationFunctionType


def build_kernel(ctx: ExitStack, tc: tile.TileContext, io: dict):
    nc = tc.nc
    x_d = io["x"]          # [N, D]   f32
    rel_d = io["rel_pos"]  # [N, N]   f32
    wq_d = io["W_qkv"]     # [E, D]   f32
    bq_d = io["b_qkv"]     # [E]      f32
    wd_d = io["W_d"]       # [D, 1]   f32
    bd_d = io["b_d"]       # [D]      f32
    out_d = io["out"]      # [N, D]   f32

    # ---------------- pools ----------------
    consts = ctx.enter_context(tc.tile_pool(name="consts", bufs=1))
    resid = ctx.enter_context(tc.tile_pool(name="resid", bufs=1))
    natbuf = ctx.enter_context(tc.tile_pool(name="natbuf", bufs=3))
    xbuf = ctx.enter_context(tc.tile_pool(name="xbuf", bufs=3))
    small = ctx.enter_context(tc.tile_pool(name="small", bufs=8))
    ptpool = ctx.enter_context(tc.tile_pool(name="ptpool", bufs=3))
    opool = ctx.enter_context(tc.tile_pool(name="opool", bufs=4))
    ps = ctx.enter_context(tc.tile_pool(name="ps", bufs=3, space="PSUM"))
    pso = ctx.enter_context(tc.tile_pool(name="pso", bufs=1, space="PSUM"))

    # ---------------- constants ----------------
    ident = consts.tile([128, 128], BF16)
    make_identity(nc, ident)

    # W_d / b_d broadcast along partitions: [128, D]
    wd_bc = consts.tile([128, D], F32)
    nc.sync.dma_start(
        out=wd_bc,
        in_=bass.AP(tensor=wd_d.tensor, offset=wd_d.offset, ap=[[0, 128], [1, D]]),
    )
    bd_bc = consts.tile([128, D], F32)
    nc.sync.dma_start(
        out=bd_bc,
        in_=bass.AP(tensor=bd_d.tensor, offset=bd_d.offset, ap=[[0, 128], [1, D]]),
    )
    # b_qkv[256:768] broadcast along partitions (bias for value/res rows)
    bvr_bc = consts.tile([128, 2 * D], F32)
    nc.sync.dma_start(
        out=bvr_bc,
        in_=bass.AP(tensor=bq_d.tensor, offset=bq_d.offset + D, ap=[[0, 128], [1, 2 * D]]),
    )
    # b_qkv[0:256] as per-partition columns [128, DC] (bias for qk chunks)
    bqkT = consts.tile([128, DC], F32)
    nc.sync.dma_start(
        out=bqkT,
        in_=bass.AP(tensor=bq_d.tensor, offset=bq_d.offset, ap=[[1, 128], [128, DC]]),
    )

    # W_qkv natural load -> bf16 -> PE-transposed WqT[dc] = W_qkv.T chunks
    wq_nat = consts.tile([128, 6, D], F32)
    nc.sync.dma_start(out=wq_nat, in_=wq_d.rearrange("(c p) d -> p c d", p=128))
    wq_nat_bf = consts.tile([128, 6, D], BF16)
    nc.vector.tensor_copy(out=wq_nat_bf, in_=wq_nat)
    wqT = [consts.tile([128, E], BF16, tag=f"wqT{dc}", name=f"wqT{dc}") for dc in range(DC)]
    for c in range(6):
        for dc in range(DC):
            pt = ps.tile([128, 128], BF16, tag="ps", name="pt_w", padded_shape=[128, 1024])
            nc.tensor.transpose(pt, wq_nat_bf[:, c, ts(dc, 128)], ident)
            nc.scalar.copy(out=wqT[dc][:, ts(c, 128)], in_=pt)

    # ---------------- resident tensors ----------------
    RT = [resid.tile([128, N], BF16, tag=f"RT{j}", name=f"RT{j}") for j in range(NT)]  # rel_pos.T
    qkT = [resid.tile([128, N], BF16, tag=f"qkT{dc}", name=f"qkT{dc}") for dc in range(DC)]
    xgT = [resid.tile([128, N], BF16, tag=f"xgT{dc}", name=f"xgT{dc}") for dc in range(DC)]
    vp = [resid.tile([128, D + 1], BF16, tag=f"vp{j}", name=f"vp{j}") for j in range(NT)]  # [V | 1]
    res = [resid.tile([128, D], F32, tag=f"res{j}", name=f"res{j}") for j in range(NT)]
    deg = resid.tile([128, NT], F32)

    # ---------------- pass A: stream rel_pos, build RT + deg + xgT ----------
    for i in range(NT):
        # HBM f32 -> SBUF bf16 cast on the software DGE
        nat = natbuf.tile([128, N], BF16, tag="nat")
        nc.gpsimd.dma_start(out=nat, in_=rel_d[ts(i, 128), :])
        # deg (f32 accumulate over bf16 row)
        nc.vector.tensor_reduce(
            out=deg[:, i : i + 1], in_=nat, axis=mybir.AxisListType.X, op=AL.add
        )
        # scatter transposed blocks into RT via the DMA XBAR (scalar HWDGE queue)
        for j in range(NT):
            nc.scalar.dma_start(
                out=RT[j][:, ts(i, 128)], in_=nat[:, ts(j, 128)], transpose=True
            )

        # x row-tile: gate + PE-transpose into xgT
        xt = xbuf.tile([128, D], F32, tag="x")
        nc.sync.dma_start(out=xt, in_=x_d[ts(i, 128), :])
        gate = xbuf.tile([128, D], F32, tag="gate")
        nc.vector.scalar_tensor_tensor(
            out=gate,
            in0=wd_bc,
            scalar=deg[:, i : i + 1],
            in1=bd_bc,
            op0=AL.mult,
            op1=AL.add,
        )
        nc.scalar.activation(out=gate, in_=gate, func=AF.Sigmoid)
        xg = xbuf.tile([128, D], BF16, tag="xg")
        nc.vector.tensor_tensor(out=xg, in0=xt, in1=gate, op=AL.mult)
        for dc in range(DC):
            pt = ps.tile([128, 128], BF16, tag="ps", name="pt_xg", padded_shape=[128, 1024])
            nc.tensor.transpose(pt, xg[:, ts(dc, 128)], ident)
            nc.scalar.copy(out=xgT[dc][:, ts(i, 128)], in_=pt)

    # ---------------- qkv projections ----------------
    # qk rows (e in [0, 256)): transposed orientation -> qkT, fused sigmoid
    for ec in range(DC):
        for g in range(NG):
            pq = ps.tile([128, 512], F32, tag="ps")
            for dc in range(DC):
                nc.tensor.matmul(
                    pq,
                    lhsT=wqT[dc][:, ts(ec, 128)],
                    rhs=xgT[dc][:, ts(g, 512)],
                    start=(dc == 0),
                    stop=(dc == DC - 1),
                )
            nc.scalar.activation(
                out=qkT[ec][:, ts(g, 512)],
                in_=pq,
                func=AF.Sigmoid,
                bias=bqkT[:, ec : ec + 1],
                scale=1.0,
            )

    # value/res rows (e in [256, 768)): natural orientation
    for i in range(NT):
        pv = ps.tile([128, 512], F32, tag="ps")
        for dc in range(DC):
            nc.tensor.matmul(
                pv,
                lhsT=xgT[dc][:, ts(i, 128)],
                rhs=wqT[dc][:, D : 3 * D],
                start=(dc == 0),
                stop=(dc == DC - 1),
            )
        nc.vector.tensor_tensor(
            out=vp[i][:, 0:D], in0=pv[:, 0:D], in1=bvr_bc[:, 0:D], op=AL.add
        )
        nc.vector.memset(vp[i][:, D : D + 1], 1.0)
        nc.vector.tensor_tensor(
            out=res[i], in0=pv[:, D : 2 * D], in1=bvr_bc[:, D : 2 * D], op=AL.add
        )

    # ---------------- attention ----------------
    for g in range(NG):
        outp = [pso.tile([128, D + 1], F32, tag=f"po{li}", name=f"po{li}") for li in range(GT)]
        for j in range(NT):
            pa = ps.tile([128, 512], F32, tag="ps")
            for dc in range(DC):
                nc.tensor.matmul(
                    pa,
                    lhsT=qkT[dc][:, ts(j, 128)],
                    rhs=qkT[dc][:, ts(g, 512)],
                    start=(dc == 0),
                    stop=(dc == DC - 1),
                )
            # P~[m, n] = (A * scale) * rel_pos.T   (bf16)
            pt = ptpool.tile([128, 512], BF16, tag="pt")
            nc.vector.scalar_tensor_tensor(
                out=pt,
                in0=pa,
                scalar=SCALE,
                in1=RT[j][:, ts(g, 512)],
                op0=AL.mult,
                op1=AL.mult,
            )
            for li in range(GT):
                nc.tensor.matmul(
                    outp[li],
                    lhsT=pt[:, ts(li, 128)],
                    rhs=vp[j],
                    start=(j == 0),
                    stop=(j == NT - 1),
                )
        for li in range(GT):
            i = g * GT + li
            z = small.tile([128, 1], F32, tag="z")
            nc.vector.tensor_scalar_add(out=z, in0=outp[li][:, D : D + 1], scalar1=EPS)
            zi = small.tile([128, 1], F32, tag="zi")
            nc.vector.reciprocal(out=zi, in_=z)
            o = opool.tile([128, D], F32, tag="o")
            nc.vector.scalar_tensor_tensor(
                out=o,
                in0=outp[li][:, 0:D],
                scalar=zi,
                in1=res[i],
                op0=AL.mult,
                op1=AL.add,
            )
            nc.scalar.activation(out=o, in_=o, func=AF.Relu)
            nc.sync.dma_start(out=out_d[ts(i, 128), :], in_=o)


_CACHE: dict = {}


def _get_nc():
    if "nc" in _CACHE:
        return _CACHE["nc"], _CACHE["io"]
    nc = bacc.Bacc("TRN2", target_bir_lowering=False, debug=False)
    io = {
        "x": nc.dram_tensor("x", [N, D], F32, kind="ExternalInput").ap(),
        "rel_pos": nc.dram_tensor("rel_pos", [N, N], F32, kind="ExternalInput").ap(),
        "W_qkv": nc.dram_tensor("W_qkv", [E, D], F32, kind="ExternalInput").ap(),
        "b_qkv": nc.dram_tensor("b_qkv", [E], F32, kind="ExternalInput").ap(),
        "W_d": nc.dram_tensor("W_d", [D, 1], F32, kind="ExternalInput").ap(),
        "b_d": nc.dram_tensor("b_d", [D], F32, kind="ExternalInput").ap(),
        "out": nc.dram_tensor("out", [N, D], F32, kind="ExternalOutput").ap(),
    }
    with tile.TileContext(nc) as tc:
        with ExitStack() as ctx:
            build_kernel(ctx, tc, io)
    nc.compile()
    _CACHE["nc"] = nc
    _CACHE["io"] = io
    return nc, io


def kernel(x, rel_pos, W_qkv, b_qkv, W_d, b_d, **run_kwargs):
    nc, _ = _get_nc()
    x = np.ascontiguousarray(np.asarray(x, dtype=np.float32))
    rel_pos = np.ascontiguousarray(np.asarray(rel_pos, dtype=np.float32))
    W_qkv = np.ascontiguousarray(np.asarray(W_qkv, dtype=np.float32))
    b_qkv = np.ascontiguousarray(np.asarray(b_qkv, dtype=np.float32))
    W_d = np.ascontiguousarray(np.asarray(W_d, dtype=np.float32))
    b_d = np.ascontiguousarray(np.asarray(b_d, dtype=np.float32))
    in_maps = [
        {
            "x": x[b],
            "rel_pos": rel_pos[b],
            "W_qkv": W_qkv,
            "b_qkv": b_qkv,
            "W_d": W_d,
            "b_d": b_d,
        }
        for b in range(B)
    ]
    r = run_bass_kernel_spmd(nc, in_maps, core_ids=list(range(B)), **run_kwargs)
    out = np.stack([r.results[b]["out"] for b in range(B)], axis=0)
    if run_kwargs:
        _CACHE["last_result"] = r
    return out


# revision 6
# speedup vs baseline: 2.5630x; 2.5630x over previous
"""Trainium2 Bass kernel for the gated-attention nn.Module.

Math (per batch element b):
    deg   = rel_pos.sum(-1)                        # [N]
    gate  = sigmoid(deg * W_d + b_d)               # [N, D]
    xg    = x * gate
    qkv   = xg @ W_qkv.T + b_qkv                   # [N, 3D]
    qk, value, res = split(qkv); qk = sigmoid(qk)
    attn  = (qk @ qk.T) * scale * rel_pos          # [N, N]
    attn  = attn / (attn.sum(-1, keepdims) + 1e-6)
    out   = relu(attn @ value + res)               # [N, D]

Sharding: pure data-parallel over batch, B == 8 == n_cores, one batch
element per NeuronCore, no collectives.

Per-core layout strategy: rel_pos is streamed once from HBM (fp32,
cast to bf16 by the SWDGE), row-sums reduced for deg, and a transposed
bf16 copy (RT = rel_pos.T) is built on-chip with the DMA XBAR so the
attention matrix can be produced directly in [m, n] (key-major)
orientation.  That orientation makes the normalizer a free extra
matmul column (ones appended to value) and needs no PE transposes of
the N x N matrix.
"""

import math
from contextlib import ExitStack

import numpy as np

import concourse.bass as bass
import concourse.tile as tile
from concourse import bacc, mybir
from concourse.bass import ts
from concourse.bass_utils import run_bass_kernel_spmd
from concourse.masks import make_identity

B, N, D = 8, 2048, 256
E = 3 * D  # 768
NT = N // 128  # 16 row tiles
DC = D // 128  # 2 dim chunks
NG = 4  # output-tile groups (each covers 512 query rows)
GT = NT // NG  # out tiles per group (4)
SCALE = 1.0 / math.sqrt(32.0)
EPS = 1e-6

F32 = mybir.dt.float32
BF16 = mybir.dt.bfloat16

AL = mybir.AluOpType
AF = mybir.ActivationFunctionType


def build_kernel(ctx: ExitStack, tc: tile.TileContext, io: dict):
    nc = tc.nc
    x_d = io["x"]          # [N, D]   f32
    rel_d = io["rel_pos"]  # [N, N]   f32
    wq_d = io["W_qkv"]     # [E, D]   f32
    bq_d = io["b_qkv"]     # [E]      f32
    wd_d = io["W_d"]       # [D, 1]   f32
    bd_d = io["b_d"]       # [D]      f32
    out_d = io["out"]      # [N, D]   f32

    # ---------------- pools ----------------
    consts = ctx.enter_context(tc.tile_pool(name="consts", bufs=1))
    resid = ctx.enter_context(tc.tile_pool(name="resid", bufs=1))
    natbuf = ctx.enter_context(tc.tile_pool(name="natbuf", bufs=3))
    xbuf = ctx.enter_context(tc.tile_pool(name="xbuf", bufs=3))
    small = ctx.enter_context(tc.tile_pool(name="small", bufs=8))
    ptpool = ctx.enter_context(tc.tile_pool(name="ptpool", bufs=3))
    opool = ctx.enter_context(tc.tile_pool(name="opool", bufs=4))
    ps = ctx.enter_context(tc.tile_pool(name="ps", bufs=3, space="PSUM"))
    pso = ctx.enter_context(tc.tile_pool(name="pso", bufs=1, space="PSUM"))

    # ---------------- constants ----------------
    ident = consts.tile([128, 128], BF16)
    make_identity(nc, ident)

    # W_d / b_d broadcast along partitions: [128, D]
    wd_bc = consts.tile([128, D], F32)
    nc.sync.dma_start(
        out=wd_bc,
        in_=bass.AP(tensor=wd_d.tensor, offset=wd_d.offset, ap=[[0, 128], [1, D]]),
    )
    bd_bc = consts.tile([128, D], F32)
    nc.sync.dma_start(
        out=bd_bc,
        in_=bass.AP(tensor=bd_d.tensor, offset=bd_d.offset, ap=[[0, 128], [1, D]]),
    )
    # b_qkv[256:768] broadcast along partitions (bias for value/res rows)
    bvr_bc = consts.tile([128, 2 * D], F32)
    nc.sync.dma_start(
        out=bvr_bc,
        in_=bass.AP(tensor=bq_d.tensor, offset=bq_d.offset + D, ap=[[0, 128], [1, 2 * D]]),
    )
    # b_qkv[0:256] as per-partition columns [128, DC] (bias for qk chunks)
    bqkT = consts.tile([128, DC], F32)
    nc.sync.dma_start(
        out=bqkT,
        in_=bass.AP(tensor=bq_d.tensor, offset=bq_d.offset, ap=[[1, 128], [128, DC]]),
    )

    # W_qkv natural load -> bf16 -> PE-transposed WqT[dc] = W_qkv.T chunks
    wq_nat = consts.tile([128, 6, D], F32)
    nc.sync.dma_start(out=wq_nat, in_=wq_d.rearrange("(c p) d -> p c d", p=128))
    wq_nat_bf = consts.tile([128, 6, D], BF16)
    nc.vector.tensor_copy(out=wq_nat_bf, in_=wq_nat)
    wqT = [consts.tile([128, E], BF16, tag=f"wqT{dc}", name=f"wqT{dc}") for dc in range(DC)]
    for c in range(6):
        for dc in range(DC):
            pt = ps.tile([128, 128], BF16, tag="ps", name="pt_w", padded_shape=[128, 1024])
            nc.tensor.transpose(pt, wq_nat_bf[:, c, ts(dc, 128)], ident)
            nc.scalar.copy(out=wqT[dc][:, ts(c, 128)], in_=pt)

    # ---------------- resident tensors ----------------
    RT = resid.tile([128, NT, N], BF16, name="RT")  # RT[p, j, n] = rel_pos[n, 128j+p]
    qkT = [resid.tile([128, N], BF16, tag=f"qkT{dc}", name=f"qkT{dc}") for dc in range(DC)]
    xgT = [resid.tile([128, N], BF16, tag=f"xgT{dc}", name=f"xgT{dc}") for dc in range(DC)]
    vp = [resid.tile([128, D + 1], BF16, tag=f"vp{j}", name=f"vp{j}") for j in range(NT)]  # [V | 1]
    res = [resid.tile([128, D], F32, tag=f"res{j}", name=f"res{j}") for j in range(NT)]
    deg = resid.tile([128, NT], F32)

    # ---------------- pass A: stream rel_pos, build RT + deg + xgT ----------
    for i in range(NT):
        # HBM f32 -> SBUF bf16 cast on the software DGE
        nat = natbuf.tile([128, N], BF16, tag="nat")
        nc.gpsimd.dma_start(out=nat, in_=rel_d[ts(i, 128), :])
        # deg (f32 accumulate over bf16 row)
        nc.vector.tensor_reduce(
            out=deg[:, i : i + 1], in_=nat, axis=mybir.AxisListType.X, op=AL.add
        )
        # one batched XBAR transpose: RT[:, :, 128i:128(i+1)] = nat blocks
        nc.scalar.dma_start(out=RT[:, :, ts(i, 128)], in_=nat, transpose=True)

        # x row-tile: gate + PE-transpose into xgT
        xt = xbuf.tile([128, D], F32, tag="x")
        nc.sync.dma_start(out=xt, in_=x_d[ts(i, 128), :])
        gate = xbuf.tile([128, D], F32, tag="gate")
        nc.vector.scalar_tensor_tensor(
            out=gate,
            in0=wd_bc,
            scalar=deg[:, i : i + 1],
            in1=bd_bc,
            op0=AL.mult,
            op1=AL.add,
        )
        nc.scalar.activation(out=gate, in_=gate, func=AF.Sigmoid)
        xg = xbuf.tile([128, D], BF16, tag="xg")
        nc.vector.tensor_tensor(out=xg, in0=xt, in1=gate, op=AL.mult)
        for dc in range(DC):
            pt = ps.tile([128, 128], BF16, tag="ps", name="pt_xg", padded_shape=[128, 1024])
            nc.tensor.transpose(pt, xg[:, ts(dc, 128)], ident)
            nc.scalar.copy(out=xgT[dc][:, ts(i, 128)], in_=pt)

    # ---------------- qkv projections ----------------
    # qk rows (e in [0, 256)): transposed orientation -> qkT, fused sigmoid
    for ec in range(DC):
        for g in range(NG):
            pq = ps.tile([128, 512], F32, tag="ps")
            for dc in range(DC):
                nc.tensor.matmul(
                    pq,
                    lhsT=wqT[dc][:, ts(ec, 128)],
                    rhs=xgT[dc][:, ts(g, 512)],
                    start=(dc == 0),
                    stop=(dc == DC - 1),
                )
            nc.scalar.activation(
                out=qkT[ec][:, ts(g, 512)],
                in_=pq,
                func=AF.Sigmoid,
                bias=bqkT[:, ec : ec + 1],
                scale=1.0,
            )

    # value/res rows (e in [256, 768)): natural orientation
    for i in range(NT):
        pv = ps.tile([128, 512], F32, tag="ps")
        for dc in range(DC):
            nc.tensor.matmul(
                pv,
                lhsT=xgT[dc][:, ts(i, 128)],
                rhs=wqT[dc][:, D : 3 * D],
                start=(dc == 0),
                stop=(dc == DC - 1),
            )
        nc.vector.tensor_tensor(
            out=vp[i][:, 0:D], in0=pv[:, 0:D], in1=bvr_bc[:, 0:D], op=AL.add
        )
        nc.vector.memset(vp[i][:, D : D + 1], 1.0)
        nc.vector.tensor_tensor(
            out=res[i], in0=pv[:, D : 2 * D], in1=bvr_bc[:, D : 2 * D], op=AL.add
        )

    # ---------------- attention ----------------
    for g in range(NG):
        outp = [pso.tile([128, D + 1], F32, tag=f"po{li}", name=f"po{li}") for li in range(GT)]
        for j in range(NT):
            pa = ps.tile([128, 512], F32, tag="ps")
            for dc in range(DC):
                nc.tensor.matmul(
                    pa,
                    lhsT=qkT[dc][:, ts(j, 128)],
                    rhs=qkT[dc][:, ts(g, 512)],
                    start=(dc == 0),
                    stop=(dc == DC - 1),
                )
            # P~[m, n] = (A * scale) * rel_pos.T   (bf16)
            pt = ptpool.tile([128, 512], BF16, tag="pt")
            nc.vector.scalar_tensor_tensor(
                out=pt,
                in0=pa,
                scalar=SCALE,
                in1=RT[:, j, ts(g, 512)],
                op0=AL.mult,
                op1=AL.mult,
            )
            for li in range(GT):
                nc.tensor.matmul(
                    outp[li],
                    lhsT=pt[:, ts(li, 128)],
                    rhs=vp[j],
                    start=(j == 0),
                    stop=(j == NT - 1),
                )
        for li in range(GT):
            i = g * GT + li
            z = small.tile([128, 1], F32, tag="z")
            nc.vector.tensor_scalar_add(out=z, in0=outp[li][:, D : D + 1], scalar1=EPS)
            zi = small.tile([128, 1], F32, tag="zi")
            nc.vector.reciprocal(out=zi, in_=z)
            o = opool.tile([128, D], F32, tag="o")
            nc.vector.scalar_tensor_tensor(
                out=o,
                in0=outp[li][:, 0:D],
                scalar=zi,
                in1=res[i],
                op0=AL.mult,
                op1=AL.add,
            )
            nc.scalar.activation(out=o, in_=o, func=AF.Relu)
            nc.sync.dma_start(out=out_d[ts(i, 128), :], in_=o)


_CACHE: dict = {}


def _get_nc():
    if "nc" in _CACHE:
        return _CACHE["nc"], _CACHE["io"]
    nc = bacc.Bacc("TRN2", target_bir_lowering=False, debug=False)
    io = {
        "x": nc.dram_tensor("x", [N, D], F32, kind="ExternalInput").ap(),
        "rel_pos": nc.dram_tensor("rel_pos", [N, N], F32, kind="ExternalInput").ap(),
        "W_qkv": nc.dram_tensor("W_qkv", [E, D], F32, kind="ExternalInput").ap(),
        "b_qkv": nc.dram_tensor("b_qkv", [E], F32, kind="ExternalInput").ap(),
        "W_d": nc.dram_tensor("W_d", [D, 1], F32, kind="ExternalInput").ap(),
        "b_d": nc.dram_tensor("b_d", [D], F32, kind="ExternalInput").ap(),
        "out": nc.dram_tensor("out", [N, D], F32, kind="ExternalOutput").ap(),
    }
    with tile.TileContext(nc) as tc:
        with ExitStack() as ctx:
            build_kernel(ctx, tc, io)
    nc.compile()
    _CACHE["nc"] = nc
    _CACHE["io"] = io
    return nc, io


def kernel(x, rel_pos, W_qkv, b_qkv, W_d, b_d, **run_kwargs):
    nc, _ = _get_nc()
    x = np.ascontiguousarray(np.asarray(x, dtype=np.float32))
    rel_pos = np.ascontiguousarray(np.asarray(rel_pos, dtype=np.float32))
    W_qkv = np.ascontiguousarray(np.asarray(W_qkv, dtype=np.float32))
    b_qkv = np.ascontiguousarray(np.asarray(b_qkv, dtype=np.float32))
    W_d = np.ascontiguousarray(np.asarray(W_d, dtype=np.float32))
    b_d = np.ascontiguousarray(np.asarray(b_d, dtype=np.float32))
    in_maps = [
        {
            "x": x[b],
            "rel_pos": rel_pos[b],
            "W_qkv": W_qkv,
            "b_qkv": b_qkv,
            "W_d": W_d,
            "b_d": b_d,
        }
        for b in range(B)
    ]
    r = run_bass_kernel_spmd(nc, in_maps, core_ids=list(range(B)), **run_kwargs)
    out = np.stack([r.results[b]["out"] for b in range(B)], axis=0)
    if run_kwargs:
        _CACHE["last_result"] = r
    return out
